# revision 2
# baseline (speedup 1.0000x reference)
"""Additive attention (B=4, Q=256, KV=1024, H=128, VS=256) on 8 Trainium2 cores.

Sharding: each core processes 32 query rows of every batch (4 groups of 32
row-slots).  Per batch, only ceil(valid_len/128) KV chunks of 128 are computed;
masked columns beyond that contribute exactly 0 to the softmax, so skipping
them is exact.  No collectives are needed.

Per-core dataflow:
  DVE   : sums[h, kv] = kp[h, kv] + qp[h, s]        (tensor_scalar add)
  ACT   : tanh in place over 8-row batches           (the throughput bottleneck)
  PE    : score rows via one-hot-wv matmuls accumulated into PSUM partitions,
          mask added with one K=4 matmul (ind ⊗ mask),
          probs transposes, final attn @ V in 32-column strips per group
  DVE   : softmax max / sum / reciprocal, final scale
"""
import math
import os
import sys

import numpy as np

for _p in ("/opt/trn_rl_repo", "/root/.axon_site/_ro/trn_rl_repo"):
    if os.path.isdir(_p):
        if _p not in sys.path:
            sys.path.insert(0, _p)
        break

B, Q, KV, QS, KS, H, VS = 4, 256, 1024, 128, 128, 128, 256
P = 128
N_CORES = 8
GROUP_ROWS = 32          # rows per (core, batch)
SUB = 8                  # rows per tanh batch

PROFILE = False          # set by test.py; enables NTFF tracing
SIMULATE = False         # set by test.py; run CoreSim instead of hardware
LAST_EXEC_NS = None

_prog_cache = {}


def _build_program(ncfg):
    """ncfg: tuple of 4 per-group KV chunk counts (sorted desc). Returns nc."""
    import contextlib

    import concourse.bacc as bacc
    import concourse.mybir as mybir
    import concourse.tile as tile

    f32 = mybir.dt.float32
    W = [c * P for c in ncfg]           # per-group computed KV width
    Wmax = W[0]
    nc = bacc.Bacc("TRN2", target_bir_lowering=False, debug=False,
                   enable_asserts=True, num_devices=N_CORES)

    qT_d = nc.dram_tensor("qT", [P, P], f32, kind="ExternalInput").ap()
    kT_d = nc.dram_tensor("kT", [P, B * KV], f32, kind="ExternalInput").ap()
    V_d = nc.dram_tensor("V", [B, KV, VS], f32, kind="ExternalInput").ap()
    Wq_d = nc.dram_tensor("Wq", [QS, H], f32, kind="ExternalInput").ap()
    Wk_d = nc.dram_tensor("Wk", [KS, H], f32, kind="ExternalInput").ap()
    wvsel_d = nc.dram_tensor("wvsel", [P, P], f32, kind="ExternalInput").ap()
    ind_d = nc.dram_tensor("ind", [B, P], f32, kind="ExternalInput").ap()
    mask_d = nc.dram_tensor("mask", [B, Wmax], f32, kind="ExternalInput").ap()
    ident_d = nc.dram_tensor("ident", [P, P], f32, kind="ExternalInput").ap()
    out_d = nc.dram_tensor("out", [P, VS], f32, kind="ExternalOutput").ap()

    with tile.TileContext(nc) as tc, contextlib.ExitStack() as ctx:
        const = ctx.enter_context(tc.tile_pool(name="const", bufs=1))
        ktp = ctx.enter_context(tc.tile_pool(name="ktp", bufs=2))
        feats_pool = ctx.enter_context(tc.tile_pool(name="featsp", bufs=3))
        small = ctx.enter_context(tc.tile_pool(name="small", bufs=1))
        psum = ctx.enter_context(tc.tile_pool(name="psum", bufs=1, space="PSUM"))
        psum2 = ctx.enter_context(tc.tile_pool(name="psum2", bufs=2, space="PSUM"))

        # ---- constant loads ----
        wq_sb = const.tile([QS, H], f32)
        nc.sync.dma_start(wq_sb[:], Wq_d[:])
        wk_sb = const.tile([KS, H], f32)
        nc.sync.dma_start(wk_sb[:], Wk_d[:])
        qt_sb = const.tile([P, P], f32)
        nc.sync.dma_start(qt_sb[:], qT_d[:])
        wvsel_sb = const.tile([P, P], f32)
        nc.sync.dma_start(wvsel_sb[:], wvsel_d[:])
        ind_sb = const.tile([B, P], f32)
        nc.sync.dma_start(ind_sb[:], ind_d[:])
        mask_sb = const.tile([B, Wmax], f32)
        nc.sync.dma_start(mask_sb[:], mask_d[:])
        ident_sb = const.tile([P, P], f32)
        nc.sync.dma_start(ident_sb[:], ident_d[:])

        # ---- projections ----
        qp_ps = psum2.tile([P, P], f32, tag="proj")
        nc.tensor.matmul(qp_ps[:], wq_sb[:], qt_sb[:], start=True, stop=True)
        qp_sb = const.tile([P, P], f32)
        nc.vector.tensor_copy(qp_sb[:], qp_ps[:])

        kp_sb = const.tile([P, B * KV], f32)
        for g in range(B):
            for j in range(0, W[g], 512):
                n = min(512, W[g] - j)
                kt_t = ktp.tile([P, 512], f32, tag="kt")
                nc.sync.dma_start(kt_t[:, :n], kT_d[:, g * KV + j: g * KV + j + n])
                kp_ps = psum2.tile([P, 512], f32, tag="proj", name=f"kp_ps_{g}_{j}")
                nc.tensor.matmul(kp_ps[:, :n], wk_sb[:], kt_t[:, :n],
                                 start=True, stop=True)
                nc.vector.tensor_copy(kp_sb[:, g * KV + j: g * KV + j + n],
                                      kp_ps[:, :n])

        # ---- one-hot lhsT tiles (2 rotating) ----
        oh = [const.tile([P, P], f32, name=f"oh{i}") for i in range(2)]
        for t in oh:
            nc.vector.memset(t[:], 0.0)

        # ---- main loop: scores ----
        scores_ps = psum.tile([P, Wmax], f32)
        started = {0: False, 512: False}
        for g in range(B):
            wg = W[g]
            for sb in range(GROUP_ROWS // SUB):
                feats = feats_pool.tile([P, SUB * wg], f32, tag="feats",
                                        name=f"feats_{g}_{sb}")
                for j in range(SUB):
                    s = GROUP_ROWS * g + SUB * sb + j
                    nc.vector.tensor_scalar_add(
                        feats[:, j * wg:(j + 1) * wg],
                        kp_sb[:, g * KV: g * KV + wg],
                        qp_sb[:, s: s + 1])
                nc.scalar.activation(feats[:], feats[:],
                                     mybir.ActivationFunctionType.Tanh)
                for j in range(SUB):
                    s = GROUP_ROWS * g + SUB * sb + j
                    t = oh[s % 2]
                    if s >= 2:
                        nc.vector.memset(t[:, s - 2: s - 1], 0.0)
                    nc.vector.tensor_copy(t[:, s: s + 1], wvsel_sb[:, s: s + 1])
                    for c0 in range(0, wg, 512):
                        c1 = min(c0 + 512, wg)
                        nc.tensor.matmul(
                            scores_ps[:, c0:c1], t[:],
                            feats[:, j * wg + c0: j * wg + c1],
                            start=not started[c0], stop=False,
                            skip_group_check=True)
                        started[c0] = True

        # ---- mask add (K=B matmul), closes the accumulation ----
        for c0 in range(0, Wmax, 512):
            c1 = min(c0 + 512, Wmax)
            nc.tensor.matmul(scores_ps[:, c0:c1], ind_sb[:], mask_sb[:, c0:c1],
                             start=False, stop=True, skip_group_check=True)

        # ---- softmax ----
        nrowmax = small.tile([P, 1], f32)
        nc.vector.reduce_max(nrowmax[:], scores_ps[:, :Wmax],
                             axis=mybir.AxisListType.X, negate=True)
        probs = small.tile([P, Wmax], f32)
        nc.scalar.activation(probs[:], scores_ps[:, :Wmax],
                             mybir.ActivationFunctionType.Exp,
                             bias=nrowmax[:, 0:1], scale=1.0)
        rowsum = small.tile([P, 1], f32)
        nc.vector.reduce_sum(rowsum[:], probs[:], axis=mybir.AxisListType.X)
        rinv = small.tile([P, 1], f32)
        nc.vector.reciprocal(rinv[:], rowsum[:])

        # ---- transpose probs chunks ----
        pts = []
        for c in range(ncfg[0]):
            pt_ps = psum2.tile([P, P], f32, tag="pt", name=f"pt_ps{c}")
            nc.tensor.transpose(pt_ps[:], probs[:, c * P:(c + 1) * P], ident_sb[:])
            pt_sb = small.tile([P, P], f32, name=f"pt_sb{c}")
            nc.vector.tensor_copy(pt_sb[:], pt_ps[:])
            pts.append(pt_sb)

        # ---- final attn @ V per group (32-column strips) ----
        out_ps = psum.tile([P, VS], f32, name="out_ps")
        for g in range(B):
            for c in range(ncfg[g]):
                v_t = const.tile([P, VS], f32, name=f"v_{g}_{c}")
                nc.sync.dma_start(v_t[:], V_d[g, c * P:(c + 1) * P, :])
                nc.tensor.matmul(
                    out_ps[GROUP_ROWS * g: GROUP_ROWS * (g + 1), :],
                    pts[c][:, GROUP_ROWS * g: GROUP_ROWS * (g + 1)], v_t[:],
                    start=(c == 0), stop=(c == ncfg[g] - 1),
                    tile_position=(0, GROUP_ROWS * g),
                    skip_group_check=True)

        out_sb = small.tile([P, VS], f32)
        nc.vector.tensor_scalar_mul(out_sb[:], out_ps[:], rinv[:, 0:1])
        nc.sync.dma_start(out_d[:], out_sb[:])

    nc.compile()
    return nc


def _get_program(ncfg):
    if ncfg not in _prog_cache:
        _prog_cache[ncfg] = _build_program(ncfg)
    return _prog_cache[ncfg]


def kernel(queries, keys, values, valid_lens, Wq, Wk, wv):
    global LAST_EXEC_NS
    queries = np.ascontiguousarray(np.asarray(queries), dtype=np.float32)
    keys = np.ascontiguousarray(np.asarray(keys), dtype=np.float32)
    values = np.ascontiguousarray(np.asarray(values), dtype=np.float32)
    Wq = np.ascontiguousarray(np.asarray(Wq), dtype=np.float32)
    Wk = np.ascontiguousarray(np.asarray(Wk), dtype=np.float32)
    wv = np.ascontiguousarray(np.asarray(wv), dtype=np.float32)
    vl = [int(x) for x in np.asarray(valid_lens)]

    nc_b = [min(8, max(1, math.ceil(L / P))) if L > 0 else 8 for L in vl]
    order = sorted(range(B), key=lambda b: (-nc_b[b], b))
    ncfg = tuple(nc_b[b] for b in order)
    Wmax = ncfg[0] * P

    nc = _get_program(ncfg)

    kT = np.concatenate([keys[order[g]].T for g in range(B)], axis=1)
    kT = np.ascontiguousarray(kT)                        # [128, 4096]
    Vm = np.ascontiguousarray(np.stack([values[order[g]] for g in range(B)]))
    wvsel = np.zeros((P, P), np.float32)
    for s in range(P):
        if vl[order[s // GROUP_ROWS]] > 0:
            wvsel[:, s] = wv
    ind = np.zeros((B, P), np.float32)
    for g in range(B):
        ind[g, GROUP_ROWS * g: GROUP_ROWS * (g + 1)] = 1.0
    mask = np.full((B, Wmax), -1e6, np.float32)
    for g in range(B):
        L = vl[order[g]]
        if L > 0:
            mask[g, :min(L, Wmax)] = 0.0
        else:
            mask[g, :] = 0.0
    ident = np.eye(P, dtype=np.float32)

    shared = {"kT": kT, "V": Vm, "Wq": Wq, "Wk": Wk, "wvsel": wvsel,
              "ind": ind, "mask": mask, "ident": ident}
    in_maps = []
    for c in range(N_CORES):
        qT = np.concatenate(
            [queries[order[g], c * GROUP_ROWS:(c + 1) * GROUP_ROWS, :].T
             for g in range(B)], axis=1)
        m = dict(shared)
        m["qT"] = np.ascontiguousarray(qT)
        in_maps.append(m)

    if SIMULATE:
        from concourse.bass_interp import CoreSim
        outs = []
        for c in range(N_CORES):
            sim = CoreSim(nc, trace=False)
            for name, v in in_maps[c].items():
                sim.tensor(name)[:] = v
            sim.simulate(check_with_hw=False)
            outs.append(sim.tensor("out").copy())
    else:
        from concourse import bass_utils
        kw = {}
        if PROFILE:
            kw = {"trace": True}
        res = bass_utils.run_bass_kernel_spmd(nc, in_maps, list(range(N_CORES)),
                                              **kw)
        if PROFILE:
            LAST_EXEC_NS = res.exec_time_ns
        outs = [res.results[c]["out"] for c in range(N_CORES)]

    out = np.zeros((B, Q, VS), np.float32)
    for c in range(N_CORES):
        for g in range(B):
            out[order[g], c * GROUP_ROWS:(c + 1) * GROUP_ROWS, :] = \
                outs[c][GROUP_ROWS * g: GROUP_ROWS * (g + 1), :]
    return out


# revision 7
# speedup vs baseline: 1.6138x; 1.6138x over previous
"""Additive attention (B=4, Q=256, KV=1024, H=128, VS=256) on 8 Trainium2 cores.

Sharding: each core processes 32 query rows of every batch (4 groups of 32
row-slots).  Per batch, only ceil(valid_len/128) KV chunks of 128 are computed;
masked columns beyond that contribute exactly 0 to the softmax, so skipping
them is exact.  No collectives are needed.

Per-core dataflow:
  DVE   : sums[h, kv] = kp[h, kv] + qp[h, s]        (tensor_scalar add)
  ACT   : tanh in place over 8-row batches           (the throughput bottleneck)
  PE    : score rows via one-hot-wv matmuls accumulated into PSUM partitions,
          mask added with one K=4 matmul (ind ⊗ mask),
          probs transposes, final attn @ V in 32-column strips per group
  DVE   : softmax max / sum / reciprocal, final scale
"""
import math
import os
import sys

import numpy as np

for _p in ("/opt/trn_rl_repo", "/root/.axon_site/_ro/trn_rl_repo"):
    if os.path.isdir(_p):
        if _p not in sys.path:
            sys.path.insert(0, _p)
        break

B, Q, KV, QS, KS, H, VS = 4, 256, 1024, 128, 128, 128, 256
P = 128
N_CORES = 8
GROUP_ROWS = 32          # rows per (core, batch)
SUB = 8                  # rows per tanh batch

PROFILE = False          # set by test.py; enables NTFF tracing
LAST_RESULTS = None
SIMULATE = False         # set by test.py; run CoreSim instead of hardware
LAST_EXEC_NS = None

_prog_cache = {}


def _build_program(cfg):
    """cfg: (ncfg, l0flags): per-group KV chunk counts (sorted desc) and
    per-group valid_len==0 flags. Returns nc."""
    ncfg, l0flags = cfg
    import contextlib

    import concourse.bacc as bacc
    import concourse.mybir as mybir
    import concourse.tile as tile

    f32 = mybir.dt.float32
    W = [c * P for c in ncfg]           # per-group computed KV width
    Wmax = W[0]
    nc = bacc.Bacc("TRN2", target_bir_lowering=False, debug=False,
                   enable_asserts=True, num_devices=N_CORES)

    qT_d = nc.dram_tensor("qT", [P, P], f32, kind="ExternalInput").ap()
    kT_d = nc.dram_tensor("kT", [P, B * KV], f32, kind="ExternalInput").ap()
    V_d = nc.dram_tensor("V", [B, KV, VS], f32, kind="ExternalInput").ap()
    Wq_d = nc.dram_tensor("Wq", [QS, H], f32, kind="ExternalInput").ap()
    Wk_d = nc.dram_tensor("Wk", [KS, H], f32, kind="ExternalInput").ap()
    wvd_d = nc.dram_tensor("wvdiag", [2, P, 2 * P - 1], mybir.dt.float32r,
                           kind="ExternalInput").ap()
    ind_d = nc.dram_tensor("ind", [B, P], f32, kind="ExternalInput").ap()
    mask_d = nc.dram_tensor("mask", [B, Wmax], f32, kind="ExternalInput").ap()
    ident_d = nc.dram_tensor("ident", [P, P], f32, kind="ExternalInput").ap()
    out_d = nc.dram_tensor("out", [P, VS], f32, kind="ExternalOutput").ap()

    with tile.TileContext(nc) as tc, contextlib.ExitStack() as ctx:
        const = ctx.enter_context(tc.tile_pool(name="const", bufs=1))
        ktp = ctx.enter_context(tc.tile_pool(name="ktp", bufs=2))
        feats_pool = ctx.enter_context(tc.tile_pool(name="featsp", bufs=3))
        small = ctx.enter_context(tc.tile_pool(name="small", bufs=1))
        psum = ctx.enter_context(tc.tile_pool(name="psum", bufs=1, space="PSUM"))
        psum2 = ctx.enter_context(tc.tile_pool(name="psum2", bufs=2, space="PSUM"))

        # ---- constant loads ----
        wq_sb = const.tile([QS, H], f32)
        nc.sync.dma_start(wq_sb[:], Wq_d[:])
        wk_sb = const.tile([KS, H], f32)
        nc.sync.dma_start(wk_sb[:], Wk_d[:])
        qt_sb = const.tile([P, P], f32)
        nc.sync.dma_start(qt_sb[:], qT_d[:])
        ind_sb = const.tile([B, P], f32)
        nc.sync.dma_start(ind_sb[:], ind_d[:])
        mask_sb = const.tile([B, Wmax], f32)
        nc.sync.dma_start(mask_sb[:], mask_d[:])
        ident_sb = const.tile([P, P], f32)
        nc.sync.dma_start(ident_sb[:], ident_d[:])

        # ---- projections ----
        qp_ps = psum2.tile([P, P], f32, tag="proj")
        nc.tensor.matmul(qp_ps[:], wq_sb[:], qt_sb[:], start=True, stop=True)
        qp_sb = const.tile([P, P], f32)
        nc.vector.tensor_copy(qp_sb[:], qp_ps[:])

        kp_sb = const.tile([P, B * KV], f32)
        for g in range(B):
            for j in range(0, W[g], 512):
                n = min(512, W[g] - j)
                kt_t = ktp.tile([P, 512], f32, tag="kt")
                nc.sync.dma_start(kt_t[:, :n], kT_d[:, g * KV + j: g * KV + j + n])
                kp_ps = psum2.tile([P, 512], f32, tag="proj", name=f"kp_ps_{g}_{j}")
                nc.tensor.matmul(kp_ps[:, :n], wk_sb[:], kt_t[:, :n],
                                 start=True, stop=True)
                nc.vector.tensor_copy(kp_sb[:, g * KV + j: g * KV + j + n],
                                      kp_ps[:, :n])

        # ---- windowed one-hot wv diagonal: wvd[:, 127-s : 255-s] has wv
        # exactly at window column s, zeros elsewhere ----
        f32r = mybir.dt.float32r
        wvd = const.tile([P, 2 * P - 1], f32r)
        nc.sync.dma_start(wvd[:], wvd_d[0])
        if any(l0flags):
            wvd0 = const.tile([P, 2 * P - 1], f32r)
            nc.sync.dma_start(wvd0[:], wvd_d[1])

        # ---- main loop: scores ----
        scores_ps = psum.tile([P, Wmax], f32)
        started = {0: False, 512: False}
        for g in range(B):
            wg = W[g]
            for sb in range(GROUP_ROWS // SUB):
                feats = feats_pool.tile([P, SUB * wg], f32r, tag="feats",
                                        name=f"feats_{g}_{sb}")
                for j in range(SUB):
                    s = GROUP_ROWS * g + SUB * sb + j
                    nc.vector.tensor_scalar_add(
                        feats[:, j * wg:(j + 1) * wg],
                        kp_sb[:, g * KV: g * KV + wg],
                        qp_sb[:, s: s + 1])
                nc.scalar.activation(feats[:], feats[:],
                                     mybir.ActivationFunctionType.Tanh)
                wsrc = wvd0 if l0flags[g] else wvd
                for j in range(SUB):
                    s = GROUP_ROWS * g + SUB * sb + j
                    for c0 in range(0, wg, 512):
                        c1 = min(c0 + 512, wg)
                        nc.tensor.matmul(
                            scores_ps[:, c0:c1],
                            wsrc[:, P - 1 - s: 2 * P - 1 - s],
                            feats[:, j * wg + c0: j * wg + c1],
                            start=not started[c0], stop=False,
                            skip_group_check=True)
                        started[c0] = True

        # ---- mask add (K=B matmul), closes the accumulation ----
        for c0 in range(0, Wmax, 512):
            c1 = min(c0 + 512, Wmax)
            nc.tensor.matmul(scores_ps[:, c0:c1], ind_sb[:], mask_sb[:, c0:c1],
                             start=False, stop=True, skip_group_check=True)

        # ---- softmax ----
        nrowmax = small.tile([P, 1], f32)
        nc.vector.reduce_max(nrowmax[:], scores_ps[:, :Wmax],
                             axis=mybir.AxisListType.X, negate=True)
        probs = small.tile([P, Wmax], f32)
        nc.scalar.activation(probs[:], scores_ps[:, :Wmax],
                             mybir.ActivationFunctionType.Exp,
                             bias=nrowmax[:, 0:1], scale=1.0)
        rowsum = small.tile([P, 1], f32)
        nc.vector.reduce_sum(rowsum[:], probs[:], axis=mybir.AxisListType.X)
        rinv = small.tile([P, 1], f32)
        nc.vector.reciprocal(rinv[:], rowsum[:])

        # ---- transpose probs chunks ----
        pts = []
        for c in range(ncfg[0]):
            pt_ps = psum2.tile([P, P], f32, tag="pt", name=f"pt_ps{c}")
            nc.tensor.transpose(pt_ps[:], probs[:, c * P:(c + 1) * P], ident_sb[:])
            pt_sb = small.tile([P, P], f32, name=f"pt_sb{c}")
            nc.vector.tensor_copy(pt_sb[:], pt_ps[:])
            pts.append(pt_sb)

        # ---- final attn @ V per group (32-column strips) ----
        out_ps = psum.tile([P, VS], f32, name="out_ps")
        for g in range(B):
            for c in range(ncfg[g]):
                v_t = const.tile([P, VS], f32, name=f"v_{g}_{c}")
                nc.sync.dma_start(v_t[:], V_d[g, c * P:(c + 1) * P, :])
                nc.tensor.matmul(
                    out_ps[GROUP_ROWS * g: GROUP_ROWS * (g + 1), :],
                    pts[c][:, GROUP_ROWS * g: GROUP_ROWS * (g + 1)], v_t[:],
                    start=(c == 0), stop=(c == ncfg[g] - 1),
                    tile_position=(0, GROUP_ROWS * g),
                    skip_group_check=True)

        out_sb = small.tile([P, VS], f32)
        nc.vector.tensor_scalar_mul(out_sb[:], out_ps[:], rinv[:, 0:1])
        nc.sync.dma_start(out_d[:], out_sb[:])

    nc.compile()
    return nc


def _get_program(ncfg):
    if ncfg not in _prog_cache:
        _prog_cache[ncfg] = _build_program(ncfg)
    return _prog_cache[ncfg]


def kernel(queries, keys, values, valid_lens, Wq, Wk, wv):
    global LAST_EXEC_NS
    queries = np.ascontiguousarray(np.asarray(queries), dtype=np.float32)
    keys = np.ascontiguousarray(np.asarray(keys), dtype=np.float32)
    values = np.ascontiguousarray(np.asarray(values), dtype=np.float32)
    Wq = np.ascontiguousarray(np.asarray(Wq), dtype=np.float32)
    Wk = np.ascontiguousarray(np.asarray(Wk), dtype=np.float32)
    wv = np.ascontiguousarray(np.asarray(wv), dtype=np.float32)
    vl = [int(x) for x in np.asarray(valid_lens)]

    nc_b = [min(8, max(1, math.ceil(L / P))) if L > 0 else 8 for L in vl]
    order = sorted(range(B), key=lambda b: (-nc_b[b], b))
    ncfg = tuple(nc_b[b] for b in order)
    l0flags = tuple(vl[order[g]] == 0 for g in range(B))
    Wmax = ncfg[0] * P

    nc = _get_program((ncfg, l0flags))

    kT = np.concatenate([keys[order[g]].T for g in range(B)], axis=1)
    kT = np.ascontiguousarray(kT)                        # [128, 4096]
    Vm = np.ascontiguousarray(np.stack([values[order[g]] for g in range(B)]))
    ind = np.zeros((B, P), np.float32)
    for g in range(B):
        ind[g, GROUP_ROWS * g: GROUP_ROWS * (g + 1)] = 1.0
    mask = np.full((B, Wmax), -1e6, np.float32)
    for g in range(B):
        L = vl[order[g]]
        if L > 0:
            mask[g, :min(L, Wmax)] = 0.0
        else:
            mask[g, :] = 0.0
    ident = np.eye(P, dtype=np.float32)

    wvdiag = np.zeros((2, P, 2 * P - 1), np.float32)
    wvdiag[0, :, P - 1] = wv
    shared = {"kT": kT, "V": Vm, "Wq": Wq, "Wk": Wk, "wvdiag": wvdiag,
              "ind": ind, "mask": mask, "ident": ident}
    in_maps = []
    for c in range(N_CORES):
        qT = np.concatenate(
            [queries[order[g], c * GROUP_ROWS:(c + 1) * GROUP_ROWS, :].T
             for g in range(B)], axis=1)
        m = dict(shared)
        m["qT"] = np.ascontiguousarray(qT)
        in_maps.append(m)

    if SIMULATE:
        from concourse.bass_interp import CoreSim
        outs = []
        for c in range(N_CORES):
            sim = CoreSim(nc, trace=False)
            for name, v in in_maps[c].items():
                sim.tensor(name)[:] = v
            sim.simulate(check_with_hw=False)
            outs.append(sim.tensor("out").copy())
    else:
        from concourse import bass_utils
        kw = {}
        if PROFILE:
            kw = {"trace": True}
        res = bass_utils.run_bass_kernel_spmd(nc, in_maps, list(range(N_CORES)),
                                              **kw)
        if PROFILE:
            LAST_EXEC_NS = res.exec_time_ns
            global LAST_RESULTS
            LAST_RESULTS = res
        outs = [res.results[c]["out"] for c in range(N_CORES)]

    out = np.zeros((B, Q, VS), np.float32)
    for c in range(N_CORES):
        for g in range(B):
            out[order[g], c * GROUP_ROWS:(c + 1) * GROUP_ROWS, :] = \
                outs[c][GROUP_ROWS * g: GROUP_ROWS * (g + 1), :]
    return out
